# revision 10
# baseline (speedup 1.0000x reference)
"""Causal self-attention (B=2, T=2048, C=2048, H=16) on 8 trn2 NeuronCores.

Sharding: core = b*4 + hg handles batch b and head-group hg (4 heads).
 - QKV projection: column-parallel over this core's 4 heads (12*128 = 1536
   output features), tokens of its batch only.
 - Attention: embarrassingly parallel over the 4 (b, h) pairs.
 - Output projection: row-parallel (this core's 512 y-channels); each core
   returns a partial [T, C] sum (bf16); the host adds the 4 partials per batch.

Key performance structure (v3):
 - Softmax denominators no longer stream every probability tile through the
   PE: probabilities are chain-accumulated in bf16 on the Vector engine
   (P_acc += pt per s-chunk), and ONE ones-stationary matmul per (head,
   q-block) reduces P_acc -> r.  This removes ~60k moving rows (~25us) from
   the Tensor engine, the kernel's roofline engine.
 - Attention runs in two passes per head (pass A: q-blocks 0-1 over s-chunks
   0..7; pass B: q-blocks 2-3 over s-chunks 0..15; union == causal set, no
   recompute).  Per pass only 2 y-accumulator PSUM banks are live, so score
   tiles get [128,1024] double-banked double-buffered PSUM and the exp runs
   as ONE ACT instruction per (head, pass, s-chunk) spanning both banks
   (fewer ACT fixed overheads; ACT paces phase 2).
 - Phase 1 starts with a v-block whose DMA (x-chunk + W_v) is interleaved at
   contraction-chunk granularity, so the PE starts ~1us in instead of ~8us.
 - Output projection shares the attention PSUM pools (no pool-close barrier
   between phases) and its per-tile copies alternate ACT/DVE.
 - All loops are ordered so consecutive matmuls share their stationary
   operand, and a post-pass on the emitted BIR deletes the redundant
   Ldweights instructions (the PE reuses the loaded weights).
 - Softmax skips the max-subtraction (logits are ~N(0, 0.8), exp is safe in
   fp32), mathematically identical to the reference.
"""

import json as _json

import numpy as np
import ml_dtypes

import bass_rust
import concourse.bass as bass
import concourse.mybir as mybir
import concourse.tile as tile
from concourse.vector_clock import ScopedClock
from concourse.bass_utils import run_bass_kernel_spmd

BF = mybir.dt.bfloat16
F32 = mybir.dt.float32
AF = mybir.ActivationFunctionType
OP = mybir.AluOpType

B, T, C = 2, 2048, 2048
H, D = 16, 128
HPC = 4          # heads per core
QB = 512         # q-block
NQB = T // QB    # 4
NJ = T // 128    # 16 s-chunks
LAG = 3          # attention pipeline lag (steps between scores and their use)
SCALE = 1.0 / float(np.sqrt(D))
N_CORES = 8


def _split_sync_waits(bir: bytes, max_waits: int = 1) -> bytes:
    """This walrus build rejects instructions carrying more than one sync
    wait (Drain takes none, DMA takes few).  Move excess waits onto NoOp
    instructions inserted immediately before the carrying instruction on the
    same engine — semantically identical, the engine just stalls at the NoOp."""
    m = _json.loads(bir)
    ctr = 0
    for fn in m["functions"]:
        for blk in fn["blocks"]:
            insts = blk.get("instructions") or []
            out = []
            for inst in insts:
                si = inst.get("sync_info")
                if si:
                    waits = si.get("on_wait") or []
                    if len(waits) > max_waits:
                        extra, keep = waits[:-max_waits], waits[-max_waits:]
                        for w in extra:
                            ctr += 1
                            out.append({
                                "debug": inst.get("debug", 0),
                                "engine": inst["engine"],
                                "ins": [],
                                "name": f"I-wsplit{ctr}",
                                "opcode": "NoOp",
                                "outs": [],
                                "sync_info": {"on_update": [], "on_wait": [w]},
                            })
                        si["on_wait"] = keep
                out.append(inst)
            blk["instructions"] = out
    return _json.dumps(m).encode()


def _dedup_ldweights(bir: bytes) -> bytes:
    """Delete PE Ldweights whose operands exactly match the previous
    Ldweights, with only Matmult/NoOp PE instructions in between (the PE
    array still holds those weights).  Sync waits on a deleted Ldweights
    move to the next kept PE instruction.  Only valid because no engine
    overwrites a stationary's SBUF region inside its reuse window."""
    m = _json.loads(bir)
    for fn in m["functions"]:
        for blk in fn["blocks"]:
            insts = blk.get("instructions") or []
            prev_key = None
            carry_waits = []
            out = []
            for inst in insts:
                if inst.get("engine") != "PE":
                    out.append(inst)
                    continue
                op = inst["opcode"]
                si = inst.get("sync_info")
                if op == "Ldweights":
                    key = _json.dumps(
                        [inst.get("ins"),
                         inst.get("perf_mode"), inst.get("is_transpose"),
                         inst.get("tile_position"), inst.get("tile_size")],
                        sort_keys=True)
                    if key == prev_key:
                        if si:
                            carry_waits.extend(si.get("on_wait") or [])
                            if si.get("on_update"):
                                # must keep an updating instruction
                                out.append(inst)
                                continue
                        continue
                    prev_key = key
                elif op in ("Matmult", "NoOp"):
                    pass
                else:
                    prev_key = None
                if carry_waits:
                    si = inst.setdefault(
                        "sync_info", {"on_update": [], "on_wait": []})
                    si["on_wait"] = carry_waits + (si.get("on_wait") or [])
                    carry_waits = []
                out.append(inst)
            blk["instructions"] = out
    return _json.dumps(m).encode()


class PatchedBass(bass.Bass):
    def to_json_bytes(self, *a, **k):
        return _split_sync_waits(_dedup_ldweights(super().to_json_bytes(*a, **k)))


class PatchedTileContext(tile.TileContext):
    """This walrus build rejects sync waits on the SP Drain (CTRL_NO_STRUCT).
    Put the end-of-kernel waits on one-wait-each NOPs ahead of the drain."""

    def _drain_and_barrier(self, tick_clock, wait_clock):
        nop0 = self.nc.sync.nop(nofuse=True)
        wait_clock.add_sem_waits(nop0.ins, ScopedClock({None: tick_clock.global_clock}))
        si = nop0.ins.sync_info
        if si is not None and len(si.on_wait) > 1:
            waits = list(si.on_wait)
            si.on_wait = waits[:1]
            for w in waits[1:]:
                n = self.nc.sync.nop(nofuse=True)
                n.ins.sync_info = bass_rust.SyncInfo(on_wait=[w], on_update=[])
        self.nc.sync.drain()
        self.nc.all_engine_barrier()
        assert self.sems is not None
        popped = self.nc._tile_sem_poison_stack.pop()
        assert popped is self._sem_poison
        self.nc.clear_and_free_semaphores(list(self.sems.allocated().values()))
        self.nc.all_engine_barrier()


def build_nc(repeat: int = 1) -> bass.Bass:
    nc = PatchedBass("TRN2", target_bir_lowering=False, debug=False)

    xT_d = nc.dram_tensor("xT", [C, T], BF, kind="ExternalInput")
    wqkv_d = nc.dram_tensor("wqkv", [C, 12 * D], BF, kind="ExternalInput")
    wp_d = nc.dram_tensor("wp", [HPC * D, C], BF, kind="ExternalInput")
    tri_d = nc.dram_tensor("tri", [128, 128], BF, kind="ExternalInput")
    out_d = nc.dram_tensor("out", [T, C], BF, kind="ExternalOutput")

    xT = xT_d.ap().rearrange("(co ci) t -> ci co t", ci=128)        # [128,16,T]
    wqkv = wqkv_d.ap().rearrange("(co ci) f -> ci co f", ci=128)    # [128,16,1536]
    wp = wp_d.ap().rearrange("(h di) c -> di h c", di=128)          # [128,4,C]

    with PatchedTileContext(nc) as tc:
      for _rep in range(repeat):
        with tc.tile_pool(name="persist", bufs=1) as persist:
            qT_sb = persist.tile([128, HPC, T], BF, tag="qT")
            kT_sb = persist.tile([128, HPC, T], BF, tag="kT")
            v_sb = persist.tile([128, NJ, HPC * D], BF, tag="v")
            yT_sb = persist.tile([128, HPC, T], BF, tag="yT")
            tri_sb = persist.tile([128, 128], BF, tag="tri")
            ones_sb = persist.tile([128, 1], BF, tag="ones")

            wp_sb = persist.tile([128, HPC, C], BF, tag="wp")
            nc.vector.memset(ones_sb, 1.0)

            # ---------------- Phase 1: QKV projection ----------------
            with tc.tile_pool(name="w1", bufs=1) as w1_pool, \
                 tc.tile_pool(name="xt", bufs=1) as xt_pool, \
                 tc.tile_pool(name="qkps", bufs=4, space="PSUM") as psqk, \
                 tc.tile_pool(name="vps", bufs=4, space="PSUM") as psv:
                w_sb = w1_pool.tile([128, 16, 12 * D], BF, tag="w")
                xts = []
                for n in range(4):
                    xt_n = xt_pool.tile([128, 16, QB], BF, tag=f"xt{n}", name=f"xt{n}")
                    xts.append(xt_n)
                # DMA schedule: v_block(0)'s inputs land first at
                # contraction-chunk granularity so the PE starts ~1us in,
                # then x1 (v_block(1) streams it), then the q/k weights.
                def xw_piece(lo, hi):
                    nc.sync.dma_start(
                        out=xts[0][:, lo:hi, :], in_=xT[:, lo:hi, 0:QB])
                    nc.sync.dma_start(
                        out=w_sb[:, lo:hi, 8 * 128:12 * 128],
                        in_=wqkv[:, lo:hi, 8 * 128:12 * 128],
                    )
                # very first pieces: just token-tile 0 of contraction chunk 0
                # plus W_v chunk 0, so the first matmul starts ~2.4us in
                nc.sync.dma_start(out=xts[0][:, 0:1, 0:128], in_=xT[:, 0:1, 0:128])
                nc.sync.dma_start(
                    out=w_sb[:, 0:1, 8 * 128:12 * 128],
                    in_=wqkv[:, 0:1, 8 * 128:12 * 128],
                )
                nc.sync.dma_start(out=xts[0][:, 0:1, 128:QB], in_=xT[:, 0:1, 128:QB])
                for c2 in range(7):
                    xw_piece(1 + 2 * c2, 3 + 2 * c2)
                xw_piece(15, 16)
                for c4 in range(4):
                    nc.sync.dma_start(
                        out=xts[1][:, 4 * c4:4 * (c4 + 1), :],
                        in_=xT[:, 4 * c4:4 * (c4 + 1), QB:2 * QB],
                    )
                nc.sync.dma_start(out=w_sb[:, :, 0:128], in_=wqkv[:, :, 0:128])
                nc.sync.dma_start(out=w_sb[:, :, 128:512], in_=wqkv[:, :, 128:512])
                nc.sync.dma_start(out=w_sb[:, :, 512:1024], in_=wqkv[:, :, 512:1024])
                nc.sync.dma_start(out=xts[2], in_=xT[:, :, 2 * QB:3 * QB])
                nc.sync.dma_start(out=xts[3], in_=xT[:, :, 3 * QB:4 * QB])
                nc.sync.dma_start(out=tri_sb, in_=tri_d.ap())
                nc.sync.dma_start(out=wp_sb, in_=wp)

                def qk_sweep(ns):
                    # stationary w[c, f-block] reused across the n's of this
                    # sweep (redundant Ldweights removed by the post-pass)
                    for f in range(8):
                        pss = {}
                        for n in ns:
                            pss[n] = psqk.tile([128, QB], F32, tag="qk",
                                               name=f"qk{n}")
                        for c in range(16):
                            for n in ns:
                                nc.tensor.matmul(
                                    pss[n],
                                    w_sb[:, c, f * 128:(f + 1) * 128],
                                    xts[n][:, c, :],
                                    start=(c == 0),
                                    stop=(c == 15),
                                )
                        dst = qT_sb if f < 4 else kT_sb
                        h = f % 4
                        for i, n in enumerate(ns):
                            dsl = dst[:, h, n * QB:(n + 1) * QB]
                            if i % 2 == 0:
                                nc.scalar.copy(out=dsl, in_=pss[n])
                            else:
                                nc.vector.tensor_copy(out=dsl, in_=pss[n])

                def v_block(n):
                    # c-outer so the startup DMA feeds the PE incrementally;
                    # all 4 token-tiles accumulate simultaneously (4 banks).
                    pss = [psv.tile([128, HPC * D], F32, tag="v",
                                    name=f"v{n}_{ti}") for ti in range(4)]
                    for c in range(16):
                        for ti in range(4):
                            nc.tensor.matmul(
                                pss[ti],
                                xts[n][:, c, ti * 128:(ti + 1) * 128],
                                w_sb[:, c, 8 * 128:12 * 128],
                                start=(c == 0),
                                stop=(c == 15),
                            )
                    for ti in range(4):
                        if ti % 2 == 0:
                            nc.vector.tensor_copy(
                                out=v_sb[:, n * 4 + ti, :], in_=pss[ti])
                        else:
                            nc.scalar.copy(
                                out=v_sb[:, n * 4 + ti, :], in_=pss[ti])

                v_block(0)
                v_block(1)
                qk_sweep([0, 1])
                qk_sweep([2, 3])
                v_block(2)
                v_block(3)

            # ------------- Phase 2: attention + Phase 3: projection -------
            # Shared pool scope: no pool-close barrier between the phases.
            with tc.tile_pool(name="scps", bufs=2, space="PSUM") as scps, \
                 tc.tile_pool(name="yps", bufs=3, space="PSUM") as yps, \
                 tc.tile_pool(name="rps", bufs=1, space="PSUM") as rps, \
                 tc.tile_pool(name="pt", bufs=LAG + 3, space="SBUF") as ptp, \
                 tc.tile_pool(name="pacc", bufs=2) as paccp, \
                 tc.tile_pool(name="rrow", bufs=2) as rrow, \
                 tc.tile_pool(name="rinvp", bufs=2) as rinvp, \
                 tc.tile_pool(name="rfull", bufs=3) as rfull, \
                 tc.tile_pool(name="osb", bufs=2) as osb:
                # one PSUM bank holds the r values: r[m] chunk k lives at
                # (partition 32k, cols 128m..128m+128), so the ln/exp run as
                # [97-partition, 128-col] ACT instructions (128 engine cycles
                # instead of 512).  Ballast partitions are memset to 1.0 once
                # (ln -> 0, exp -> 1, never read).  Rows are reused across
                # heads; the ln read drains long before the next head's
                # write reaches the same diag step.
                r_ps = rps.tile([128, QB], F32, tag="r")
                nc.vector.memset(r_ps, 1.0)

                def make_pass(h, base, njp):
                    return {"h": h, "base": base, "njp": njp,
                            "y": None, "pacc": None, "pts": {}}

                def sc_bounds(st, j):
                    jm = j // 4
                    if jm >= st["base"]:
                        return jm, (jm - st["base"]) * 512 + 128 * (j % 4)
                    return st["base"], 0

                def do_scores(st, j):
                    h, base = st["h"], st["base"]
                    if j == 0:
                        st["y"] = {m: yps.tile([128, QB], F32, tag="y",
                                               name=f"y{h}_{m}")
                                   for m in (base, base + 1)}
                        st["pacc"] = paccp.tile([128, 1024], BF, tag="pa", name=f"pa{h}_{base}")
                    pacc = st["pacc"]
                    jm, lo = sc_bounds(st, j)
                    sc = scps.tile([128, 1024], F32, tag="sc", name=f"sc{h}_{j}")
                    for m in range(jm, base + 2):
                        s0 = (m - base) * 512 + (lo % 512 if m == jm else 0)
                        nc.tensor.matmul(
                            sc[:, s0:(m - base + 1) * 512],
                            kT_sb[:, h, j * 128:(j + 1) * 128],
                            qT_sb[:, h, m * 512 + (s0 - (m - base) * 512):(m + 1) * 512],
                            start=True, stop=True,
                        )
                    pt = ptp.tile([128, 1024], BF, tag="pt", name=f"pt{h}_{j}")
                    nc.scalar.activation(
                        out=pt[:, lo:1024], in_=sc[:, lo:1024],
                        func=AF.Exp, scale=SCALE,
                    )
                    if jm == j // 4 and j // 4 >= base:
                        nc.vector.tensor_tensor(
                            out=pt[:, lo:lo + 128], in0=pt[:, lo:lo + 128],
                            in1=tri_sb, op=OP.mult,
                        )
                    if j == 0:
                        nc.vector.tensor_copy(
                            out=pacc[:, lo:1024], in_=pt[:, lo:1024])
                    else:
                        nc.vector.tensor_tensor(
                            out=pacc[:, lo:1024], in0=pacc[:, lo:1024],
                            in1=pt[:, lo:1024], op=OP.add,
                        )
                    st["pts"][j] = (pt, jm, lo)

                def finish_m(st, m):
                    h, base = st["h"], st["base"]
                    mc = slice(128 * m, 128 * (m + 1))
                    for k in range(4):
                        nc.tensor.matmul(
                            r_ps[32 * k:32 * k + 1, mc],
                            ones_sb,
                            st["pacc"][:, (m - base) * 512 + 128 * k:
                                       (m - base) * 512 + 128 * (k + 1)],
                            start=True, stop=True,
                            tile_position=(0, 32 * k),
                        )
                    lnr = rrow.tile([128, 128], F32, tag="rr", name=f"rr{h}_{m}")
                    nc.scalar.activation(
                        out=lnr[0:97, :], in_=r_ps[0:97, mc], func=AF.Ln)
                    rinv = rinvp.tile([128, 128], BF, tag="ri", name=f"ri{h}_{m}")
                    nc.scalar.activation(
                        out=rinv[0:97, :], in_=lnr[0:97, :], func=AF.Exp,
                        scale=-1.0)
                    rf = rfull.tile([128, QB], BF, tag="rf", name=f"rf{h}_{m}")
                    for k in range(4):
                        rsrc = rinv[32 * k:32 * k + 1, :]
                        rinv_bcast = bass.AP(
                            tensor=rsrc.tensor,
                            offset=rsrc.offset,
                            ap=[list(rsrc.ap[0]), [0, 128]] + list(rsrc.ap[-1:]),
                        )
                        nc.sync.dma_start(
                            out=rf[:, 128 * k:128 * (k + 1)], in_=rinv_bcast)
                    nc.vector.tensor_tensor(
                        out=yT_sb[:, h, m * QB:(m + 1) * QB],
                        in0=st["y"][m], in1=rf, op=OP.mult,
                    )

                def do_av(st, j):
                    h, base = st["h"], st["base"]
                    pt, jm, lo = st["pts"].pop(j)
                    for m in range(jm, base + 2):
                        s0 = (m - base) * 512 + (lo % 512 if m == jm else 0)
                        off_m = s0 - (m - base) * 512
                        nc.tensor.matmul(
                            st["y"][m][:, off_m:QB],
                            v_sb[:, j, h * D:(h + 1) * D],
                            pt[:, s0:(m - base + 1) * 512],
                            start=(j == 0),
                            stop=(j == 4 * m + 3),
                        )
                    if jm == j // 4 and j == 4 * jm + 3:
                        finish_m(st, jm)

                # one flat software pipeline over every (head, pass, j) unit:
                # no per-pass or per-head drain/refill.
                units = []
                for h in range(HPC):
                    for base, njp in ((0, 8), (2, 16)):
                        st = make_pass(h, base, njp)
                        for j in range(njp):
                            units.append((st, j))
                n_units = len(units)
                for step in range(n_units + LAG):
                    if step >= LAG:
                        do_av(*units[step - LAG])
                    if step < n_units:
                        do_scores(*units[step])

                # ---------------- Phase 3: output projection ----------------
                for t in range(T // 128):
                    o01 = scps.tile([128, 1024], F32, tag="sc", name=f"o01_{t}")
                    o2 = yps.tile([128, QB], F32, tag="y", name=f"o2_{t}")
                    o3 = yps.tile([128, QB], F32, tag="y", name=f"o3_{t}")
                    slices = [o01[:, 0:512], o01[:, 512:1024], o2, o3]
                    for hh in range(HPC):
                        for cc in range(4):
                            nc.tensor.matmul(
                                slices[cc],
                                yT_sb[:, hh, t * 128:(t + 1) * 128],
                                wp_sb[:, hh, cc * 512:(cc + 1) * 512],
                                start=(hh == 0),
                                stop=(hh == HPC - 1),
                            )
                    ot = osb.tile([128, C], BF, tag="ot", name=f"ot{t}")
                    orow = out_d.ap()[t * 128:(t + 1) * 128, :]
                    if t == T // 128 - 1:
                        # last tile: split copies across both engines and DMA
                        # per 512-col chunk so the drain tail is short
                        nc.scalar.copy(out=ot[:, 0:512], in_=o01[:, 0:512])
                        nc.sync.dma_start(out=orow[:, 0:512], in_=ot[:, 0:512])
                        nc.vector.tensor_copy(out=ot[:, 512:1024], in_=o01[:, 512:1024])
                        nc.sync.dma_start(out=orow[:, 512:1024], in_=ot[:, 512:1024])
                        nc.scalar.copy(out=ot[:, 1024:1536], in_=o2)
                        nc.sync.dma_start(out=orow[:, 1024:1536], in_=ot[:, 1024:1536])
                        nc.vector.tensor_copy(out=ot[:, 1536:2048], in_=o3)
                        nc.sync.dma_start(out=orow[:, 1536:2048], in_=ot[:, 1536:2048])
                    else:
                        if t % 2 == 0:
                            nc.scalar.copy(out=ot[:, 0:1024], in_=o01)
                            nc.vector.tensor_copy(out=ot[:, 1024:1536], in_=o2)
                            nc.vector.tensor_copy(out=ot[:, 1536:2048], in_=o3)
                        else:
                            nc.vector.tensor_copy(out=ot[:, 0:1024], in_=o01)
                            nc.scalar.copy(out=ot[:, 1024:1536], in_=o2)
                            nc.scalar.copy(out=ot[:, 1536:2048], in_=o3)
                        nc.sync.dma_start(out=orow[:, 0:1024], in_=ot[:, 0:1024])
                        nc.sync.dma_start(out=orow[:, 1024:2048], in_=ot[:, 1024:2048])
    return nc


_NC = None


def _get_nc():
    global _NC
    if _NC is None:
        _NC = build_nc()
    return _NC


def make_in_maps(x, W_attn, W_proj):
    """Host-side sharding/layout prep. Returns list of 8 per-core input dicts."""
    bf = ml_dtypes.bfloat16
    x2 = np.asarray(x, dtype=np.float32)
    xT = np.ascontiguousarray(np.transpose(x2, (0, 2, 1))).astype(bf)  # [B, C, T]

    W = np.asarray(W_attn, dtype=np.float32)
    Wq, Wk, Wv = W[:, :C], W[:, C:2 * C], W[:, 2 * C:]
    Wp = np.asarray(W_proj, dtype=np.float32)

    # single 128x128 staircase mask: tri[s, q] = (q >= s)
    s_rel = np.arange(128)[:, None]
    q_rel = np.arange(128)[None, :]
    tri = (q_rel >= s_rel).astype(np.float32).astype(bf)  # [128, 128]

    in_maps = []
    for core in range(N_CORES):
        b, hg = core // HPC, core % HPC
        fs = slice(hg * HPC * D, (hg + 1) * HPC * D)   # this core's 512 channels
        wqkv = np.ascontiguousarray(
            np.concatenate([Wq[:, fs], Wk[:, fs], Wv[:, fs]], axis=1)
        ).astype(bf)                                    # [C, 1536]
        wp_l = np.ascontiguousarray(Wp[fs, :]).astype(bf)  # [512, C]
        in_maps.append({
            "xT": np.ascontiguousarray(xT[b]),
            "wqkv": wqkv,
            "wp": wp_l,
            "tri": np.ascontiguousarray(tri),
        })
    return in_maps


def combine_outputs(outs):
    """Sum the 4 per-head-group bf16 partials for each batch; stack to [B, T, C]."""
    out = np.empty((B, T, C), dtype=np.float32)
    for b in range(B):
        acc = outs[b * HPC].astype(np.float32)
        for hg in range(1, HPC):
            acc += outs[b * HPC + hg].astype(np.float32)
        out[b] = acc
    return out


def kernel(x, W_attn, W_proj, mask=None):
    in_maps = make_in_maps(x, W_attn, W_proj)
    nc = _get_nc()
    res = run_bass_kernel_spmd(nc, in_maps, core_ids=list(range(N_CORES)))
    outs = [r["out"] for r in res.results]
    return combine_outputs(outs)


if __name__ == "__main__":
    rng = np.random.default_rng(0)
    x = rng.standard_normal((B, T, C), dtype=np.float32)
    W_attn = rng.standard_normal((C, 3 * C), dtype=np.float32) * 0.02
    W_proj = rng.standard_normal((C, C), dtype=np.float32) * 0.02
    out = kernel(x, W_attn, W_proj)
    print("out", out.shape, out.dtype, np.abs(out).max())


# revision 12
# speedup vs baseline: 1.0169x; 1.0169x over previous
"""Causal self-attention (B=2, T=2048, C=2048, H=16) on 8 trn2 NeuronCores.

Sharding: core = b*4 + hg handles batch b and head-group hg (4 heads).
 - QKV projection: column-parallel over this core's 4 heads (12*128 = 1536
   output features), tokens of its batch only.
 - Attention: embarrassingly parallel over the 4 (b, h) pairs.
 - Output projection: row-parallel (this core's 512 y-channels); each core
   returns a partial [T, C] sum (bf16); the host adds the 4 partials per batch.

Key performance structure (v3):
 - Softmax denominators no longer stream every probability tile through the
   PE: probabilities are chain-accumulated in bf16 on the Vector engine
   (P_acc += pt per s-chunk), and ONE ones-stationary matmul per (head,
   q-block) reduces P_acc -> r.  This removes ~60k moving rows (~25us) from
   the Tensor engine, the kernel's roofline engine.
 - Attention runs in two passes per head (pass A: q-blocks 0-1 over s-chunks
   0..7; pass B: q-blocks 2-3 over s-chunks 0..15; union == causal set, no
   recompute).  Per pass only 2 y-accumulator PSUM banks are live, so score
   tiles get [128,1024] double-banked double-buffered PSUM and the exp runs
   as ONE ACT instruction per (head, pass, s-chunk) spanning both banks
   (fewer ACT fixed overheads; ACT paces phase 2).
 - Phase 1 starts with a v-block whose DMA (x-chunk + W_v) is interleaved at
   contraction-chunk granularity, so the PE starts ~1us in instead of ~8us.
 - Output projection shares the attention PSUM pools (no pool-close barrier
   between phases) and its per-tile copies alternate ACT/DVE.
 - All loops are ordered so consecutive matmuls share their stationary
   operand, and a post-pass on the emitted BIR deletes the redundant
   Ldweights instructions (the PE reuses the loaded weights).
 - Softmax skips the max-subtraction (logits are ~N(0, 0.8), exp is safe in
   fp32), mathematically identical to the reference.
"""

import json as _json

import numpy as np
import ml_dtypes

import bass_rust
import concourse.bass as bass
import concourse.mybir as mybir
import concourse.tile as tile
from concourse.vector_clock import ScopedClock
from concourse.bass_utils import run_bass_kernel_spmd

BF = mybir.dt.bfloat16
F32 = mybir.dt.float32
AF = mybir.ActivationFunctionType
OP = mybir.AluOpType

B, T, C = 2, 2048, 2048
H, D = 16, 128
HPC = 4          # heads per core
QB = 512         # q-block
NQB = T // QB    # 4
NJ = T // 128    # 16 s-chunks
LAG = 3          # attention pipeline lag (steps between scores and their use)
SCALE = 1.0 / float(np.sqrt(D))
N_CORES = 8


def _split_sync_waits(bir: bytes, max_waits: int = 1) -> bytes:
    """This walrus build rejects instructions carrying more than one sync
    wait (Drain takes none, DMA takes few).  Move excess waits onto NoOp
    instructions inserted immediately before the carrying instruction on the
    same engine — semantically identical, the engine just stalls at the NoOp."""
    m = _json.loads(bir)
    ctr = 0
    for fn in m["functions"]:
        for blk in fn["blocks"]:
            insts = blk.get("instructions") or []
            out = []
            for inst in insts:
                si = inst.get("sync_info")
                if si:
                    waits = si.get("on_wait") or []
                    if len(waits) > max_waits:
                        extra, keep = waits[:-max_waits], waits[-max_waits:]
                        for w in extra:
                            ctr += 1
                            out.append({
                                "debug": inst.get("debug", 0),
                                "engine": inst["engine"],
                                "ins": [],
                                "name": f"I-wsplit{ctr}",
                                "opcode": "NoOp",
                                "outs": [],
                                "sync_info": {"on_update": [], "on_wait": [w]},
                            })
                        si["on_wait"] = keep
                out.append(inst)
            blk["instructions"] = out
    return _json.dumps(m).encode()


def _dedup_ldweights(bir: bytes) -> bytes:
    """Delete PE Ldweights whose operands exactly match the previous
    Ldweights, with only Matmult/NoOp PE instructions in between (the PE
    array still holds those weights).  Sync waits on a deleted Ldweights
    move to the next kept PE instruction.  Only valid because no engine
    overwrites a stationary's SBUF region inside its reuse window."""
    m = _json.loads(bir)
    for fn in m["functions"]:
        for blk in fn["blocks"]:
            insts = blk.get("instructions") or []
            prev_key = None
            carry_waits = []
            out = []
            for inst in insts:
                if inst.get("engine") != "PE":
                    out.append(inst)
                    continue
                op = inst["opcode"]
                si = inst.get("sync_info")
                if op == "Ldweights":
                    key = _json.dumps(
                        [inst.get("ins"),
                         inst.get("perf_mode"), inst.get("is_transpose"),
                         inst.get("tile_position"), inst.get("tile_size")],
                        sort_keys=True)
                    if key == prev_key:
                        if si:
                            carry_waits.extend(si.get("on_wait") or [])
                            if si.get("on_update"):
                                # must keep an updating instruction
                                out.append(inst)
                                continue
                        continue
                    prev_key = key
                elif op in ("Matmult", "NoOp"):
                    pass
                else:
                    prev_key = None
                if carry_waits:
                    si = inst.setdefault(
                        "sync_info", {"on_update": [], "on_wait": []})
                    si["on_wait"] = carry_waits + (si.get("on_wait") or [])
                    carry_waits = []
                out.append(inst)
            blk["instructions"] = out
    return _json.dumps(m).encode()


class PatchedBass(bass.Bass):
    def to_json_bytes(self, *a, **k):
        return _split_sync_waits(_dedup_ldweights(super().to_json_bytes(*a, **k)))


class PatchedTileContext(tile.TileContext):
    """This walrus build rejects sync waits on the SP Drain (CTRL_NO_STRUCT).
    Put the end-of-kernel waits on one-wait-each NOPs ahead of the drain."""

    def _drain_and_barrier(self, tick_clock, wait_clock):
        nop0 = self.nc.sync.nop(nofuse=True)
        wait_clock.add_sem_waits(nop0.ins, ScopedClock({None: tick_clock.global_clock}))
        si = nop0.ins.sync_info
        if si is not None and len(si.on_wait) > 1:
            waits = list(si.on_wait)
            si.on_wait = waits[:1]
            for w in waits[1:]:
                n = self.nc.sync.nop(nofuse=True)
                n.ins.sync_info = bass_rust.SyncInfo(on_wait=[w], on_update=[])
        self.nc.sync.drain()
        self.nc.all_engine_barrier()
        assert self.sems is not None
        popped = self.nc._tile_sem_poison_stack.pop()
        assert popped is self._sem_poison
        self.nc.clear_and_free_semaphores(list(self.sems.allocated().values()))
        self.nc.all_engine_barrier()


def build_nc(repeat: int = 1) -> bass.Bass:
    nc = PatchedBass("TRN2", target_bir_lowering=False, debug=False)

    xT_d = nc.dram_tensor("xT", [C, T], BF, kind="ExternalInput")
    wqkv_d = nc.dram_tensor("wqkv", [C, 12 * D], BF, kind="ExternalInput")
    wp_d = nc.dram_tensor("wp", [HPC * D, C], BF, kind="ExternalInput")
    tri_d = nc.dram_tensor("tri", [128, 128], BF, kind="ExternalInput")
    out_d = nc.dram_tensor("out", [T, C], BF, kind="ExternalOutput")

    xT = xT_d.ap().rearrange("(co ci) t -> ci co t", ci=128)        # [128,16,T]
    wqkv = wqkv_d.ap().rearrange("(co ci) f -> ci co f", ci=128)    # [128,16,1536]
    wp = wp_d.ap().rearrange("(h di) c -> di h c", di=128)          # [128,4,C]

    with PatchedTileContext(nc) as tc:
      for _rep in range(repeat):
        with tc.tile_pool(name="persist", bufs=1) as persist:
            qT_sb = persist.tile([128, HPC, T], BF, tag="qT")
            kT_sb = persist.tile([128, HPC, T], BF, tag="kT")
            v_sb = persist.tile([128, NJ, HPC * D], BF, tag="v")
            yT_sb = persist.tile([128, HPC, T], BF, tag="yT")
            tri_sb = persist.tile([128, 128], BF, tag="tri")
            ones_sb = persist.tile([128, 1], BF, tag="ones")

            wp_sb = persist.tile([128, HPC, C], BF, tag="wp")
            nc.vector.memset(ones_sb, 1.0)

            # ---------------- Phase 1: QKV projection ----------------
            with tc.tile_pool(name="w1", bufs=1) as w1_pool, \
                 tc.tile_pool(name="xt", bufs=1) as xt_pool, \
                 tc.tile_pool(name="p1ps", bufs=8, space="PSUM") as ps1:
                w_sb = w1_pool.tile([128, 16, 12 * D], BF, tag="w")
                xts = []
                for n in range(4):
                    xt_n = xt_pool.tile([128, 16, QB], BF, tag=f"xt{n}", name=f"xt{n}")
                    xts.append(xt_n)
                # DMA schedule: v_block(0)'s inputs land first at
                # contraction-chunk granularity so the PE starts ~1us in,
                # then x1 (v_block(1) streams it), then the q/k weights.
                def xw_piece(lo, hi):
                    nc.sync.dma_start(
                        out=xts[0][:, lo:hi, :], in_=xT[:, lo:hi, 0:QB])
                    nc.sync.dma_start(
                        out=w_sb[:, lo:hi, 8 * 128:12 * 128],
                        in_=wqkv[:, lo:hi, 8 * 128:12 * 128],
                    )
                # very first pieces: just token-tile 0 of contraction chunk 0
                # plus W_v chunk 0, so the first matmul starts ~2.4us in
                nc.sync.dma_start(out=xts[0][:, 0:1, 0:128], in_=xT[:, 0:1, 0:128])
                nc.sync.dma_start(
                    out=w_sb[:, 0:1, 8 * 128:12 * 128],
                    in_=wqkv[:, 0:1, 8 * 128:12 * 128],
                )
                nc.sync.dma_start(out=xts[0][:, 0:1, 128:QB], in_=xT[:, 0:1, 128:QB])
                for c2 in range(7):
                    xw_piece(1 + 2 * c2, 3 + 2 * c2)
                xw_piece(15, 16)
                for c4 in range(4):
                    nc.sync.dma_start(
                        out=xts[1][:, 4 * c4:4 * (c4 + 1), :],
                        in_=xT[:, 4 * c4:4 * (c4 + 1), QB:2 * QB],
                    )
                nc.sync.dma_start(out=w_sb[:, :, 0:128], in_=wqkv[:, :, 0:128])
                nc.sync.dma_start(out=xts[2], in_=xT[:, :, 2 * QB:3 * QB])
                for c4 in range(4):
                    nc.sync.dma_start(
                        out=xts[3][:, 4 * c4:4 * (c4 + 1), :],
                        in_=xT[:, 4 * c4:4 * (c4 + 1), 3 * QB:4 * QB],
                    )
                nc.sync.dma_start(out=w_sb[:, :, 128:512], in_=wqkv[:, :, 128:512])
                nc.sync.dma_start(out=w_sb[:, :, 512:1024], in_=wqkv[:, :, 512:1024])
                nc.sync.dma_start(out=tri_sb, in_=tri_d.ap())
                nc.sync.dma_start(out=wp_sb, in_=wp)

                def qk_sweep(ns):
                    # stationary w[c, f-block] reused across ALL n's of the
                    # sweep (redundant Ldweights removed by the post-pass):
                    # one weight load per 4x512 moving rows.
                    for f in range(8):
                        pss = {}
                        for n in ns:
                            pss[n] = ps1.tile([128, QB], F32, tag="p1",
                                              name=f"qk{f}_{n}")
                        for c in range(16):
                            for n in ns:
                                nc.tensor.matmul(
                                    pss[n],
                                    w_sb[:, c, f * 128:(f + 1) * 128],
                                    xts[n][:, c, :],
                                    start=(c == 0),
                                    stop=(c == 15),
                                )
                        dst = qT_sb if f < 4 else kT_sb
                        h = f % 4
                        for i, n in enumerate(ns):
                            dsl = dst[:, h, n * QB:(n + 1) * QB]
                            if i % 2 == 0:
                                nc.scalar.copy(out=dsl, in_=pss[n])
                            else:
                                nc.vector.tensor_copy(out=dsl, in_=pss[n])

                def v_block(n):
                    # c-outer so the startup DMA feeds the PE incrementally;
                    # all 4 token-tiles accumulate simultaneously (4 banks).
                    pss = [ps1.tile([128, HPC * D], F32, tag="p1",
                                    name=f"v{n}_{ti}") for ti in range(4)]
                    for c in range(16):
                        for ti in range(4):
                            nc.tensor.matmul(
                                pss[ti],
                                xts[n][:, c, ti * 128:(ti + 1) * 128],
                                w_sb[:, c, 8 * 128:12 * 128],
                                start=(c == 0),
                                stop=(c == 15),
                            )
                    for ti in range(4):
                        if ti % 2 == 0:
                            nc.vector.tensor_copy(
                                out=v_sb[:, n * 4 + ti, :], in_=pss[ti])
                        else:
                            nc.scalar.copy(
                                out=v_sb[:, n * 4 + ti, :], in_=pss[ti])

                v_block(0)
                v_block(1)
                qk_sweep([0, 1, 2, 3])
                v_block(2)
                v_block(3)

            # ------------- Phase 2: attention + Phase 3: projection -------
            # Shared pool scope: no pool-close barrier between the phases.
            with tc.tile_pool(name="scps", bufs=2, space="PSUM") as scps, \
                 tc.tile_pool(name="yps", bufs=3, space="PSUM") as yps, \
                 tc.tile_pool(name="rps", bufs=1, space="PSUM") as rps, \
                 tc.tile_pool(name="pt", bufs=LAG + 3, space="SBUF") as ptp, \
                 tc.tile_pool(name="pacc", bufs=2) as paccp, \
                 tc.tile_pool(name="rrow", bufs=2) as rrow, \
                 tc.tile_pool(name="rinvp", bufs=2) as rinvp, \
                 tc.tile_pool(name="rfull", bufs=3) as rfull, \
                 tc.tile_pool(name="osb", bufs=2) as osb:
                # one PSUM bank holds the r values: r[m] chunk k lives at
                # (partition 32k, cols 128m..128m+128), so the ln/exp run as
                # [97-partition, 128-col] ACT instructions (128 engine cycles
                # instead of 512).  Ballast partitions are memset to 1.0 once
                # (ln -> 0, exp -> 1, never read).  Rows are reused across
                # heads; the ln read drains long before the next head's
                # write reaches the same diag step.
                r_ps = rps.tile([128, QB], F32, tag="r")
                nc.vector.memset(r_ps, 1.0)

                def make_pass(h, base, njp):
                    return {"h": h, "base": base, "njp": njp,
                            "y": None, "pacc": None, "pts": {}}

                def sc_bounds(st, j):
                    jm = j // 4
                    if jm >= st["base"]:
                        return jm, (jm - st["base"]) * 512 + 128 * (j % 4)
                    return st["base"], 0

                def do_scores(st, j):
                    h, base = st["h"], st["base"]
                    if j == 0:
                        st["y"] = {m: yps.tile([128, QB], F32, tag="y",
                                               name=f"y{h}_{m}")
                                   for m in (base, base + 1)}
                        st["pacc"] = paccp.tile([128, 1024], BF, tag="pa", name=f"pa{h}_{base}")
                    pacc = st["pacc"]
                    jm, lo = sc_bounds(st, j)
                    sc = scps.tile([128, 1024], F32, tag="sc", name=f"sc{h}_{j}")
                    for m in range(jm, base + 2):
                        s0 = (m - base) * 512 + (lo % 512 if m == jm else 0)
                        nc.tensor.matmul(
                            sc[:, s0:(m - base + 1) * 512],
                            kT_sb[:, h, j * 128:(j + 1) * 128],
                            qT_sb[:, h, m * 512 + (s0 - (m - base) * 512):(m + 1) * 512],
                            start=True, stop=True,
                        )
                    pt = ptp.tile([128, 1024], BF, tag="pt", name=f"pt{h}_{j}")
                    nc.scalar.activation(
                        out=pt[:, lo:1024], in_=sc[:, lo:1024],
                        func=AF.Exp, scale=SCALE,
                    )
                    if jm == j // 4 and j // 4 >= base:
                        nc.vector.tensor_tensor(
                            out=pt[:, lo:lo + 128], in0=pt[:, lo:lo + 128],
                            in1=tri_sb, op=OP.mult,
                        )
                    if j == 0:
                        nc.vector.tensor_copy(
                            out=pacc[:, lo:1024], in_=pt[:, lo:1024])
                    else:
                        nc.vector.tensor_tensor(
                            out=pacc[:, lo:1024], in0=pacc[:, lo:1024],
                            in1=pt[:, lo:1024], op=OP.add,
                        )
                    st["pts"][j] = (pt, jm, lo)

                def finish_m(st, m):
                    h, base = st["h"], st["base"]
                    mc = slice(128 * m, 128 * (m + 1))
                    for k in range(4):
                        nc.tensor.matmul(
                            r_ps[32 * k:32 * k + 1, mc],
                            ones_sb,
                            st["pacc"][:, (m - base) * 512 + 128 * k:
                                       (m - base) * 512 + 128 * (k + 1)],
                            start=True, stop=True,
                            tile_position=(0, 32 * k),
                        )
                    lnr = rrow.tile([128, 128], F32, tag="rr", name=f"rr{h}_{m}")
                    nc.scalar.activation(
                        out=lnr[0:97, :], in_=r_ps[0:97, mc], func=AF.Ln)
                    rinv = rinvp.tile([128, 128], BF, tag="ri", name=f"ri{h}_{m}")
                    nc.scalar.activation(
                        out=rinv[0:97, :], in_=lnr[0:97, :], func=AF.Exp,
                        scale=-1.0)
                    rf = rfull.tile([128, QB], BF, tag="rf", name=f"rf{h}_{m}")
                    for k in range(4):
                        rsrc = rinv[32 * k:32 * k + 1, :]
                        rinv_bcast = bass.AP(
                            tensor=rsrc.tensor,
                            offset=rsrc.offset,
                            ap=[list(rsrc.ap[0]), [0, 128]] + list(rsrc.ap[-1:]),
                        )
                        nc.sync.dma_start(
                            out=rf[:, 128 * k:128 * (k + 1)], in_=rinv_bcast)
                    nc.vector.tensor_tensor(
                        out=yT_sb[:, h, m * QB:(m + 1) * QB],
                        in0=st["y"][m], in1=rf, op=OP.mult,
                    )

                def do_av(st, j):
                    h, base = st["h"], st["base"]
                    pt, jm, lo = st["pts"].pop(j)
                    for m in range(jm, base + 2):
                        s0 = (m - base) * 512 + (lo % 512 if m == jm else 0)
                        off_m = s0 - (m - base) * 512
                        nc.tensor.matmul(
                            st["y"][m][:, off_m:QB],
                            v_sb[:, j, h * D:(h + 1) * D],
                            pt[:, s0:(m - base + 1) * 512],
                            start=(j == 0),
                            stop=(j == 4 * m + 3),
                        )
                    if jm == j // 4 and j == 4 * jm + 3:
                        finish_m(st, jm)

                # one flat software pipeline over every (head, pass, j) unit:
                # no per-pass or per-head drain/refill.
                units = []
                for h in range(HPC):
                    for base, njp in ((0, 8), (2, 16)):
                        st = make_pass(h, base, njp)
                        for j in range(njp):
                            units.append((st, j))
                n_units = len(units)
                for step in range(n_units + LAG):
                    if step >= LAG:
                        do_av(*units[step - LAG])
                    if step < n_units:
                        do_scores(*units[step])

                # ---------------- Phase 3: output projection ----------------
                for t in range(T // 128):
                    o01 = scps.tile([128, 1024], F32, tag="sc", name=f"o01_{t}")
                    o2 = yps.tile([128, QB], F32, tag="y", name=f"o2_{t}")
                    o3 = yps.tile([128, QB], F32, tag="y", name=f"o3_{t}")
                    slices = [o01[:, 0:512], o01[:, 512:1024], o2, o3]
                    for hh in range(HPC):
                        for cc in range(4):
                            nc.tensor.matmul(
                                slices[cc],
                                yT_sb[:, hh, t * 128:(t + 1) * 128],
                                wp_sb[:, hh, cc * 512:(cc + 1) * 512],
                                start=(hh == 0),
                                stop=(hh == HPC - 1),
                            )
                    ot = osb.tile([128, C], BF, tag="ot", name=f"ot{t}")
                    orow = out_d.ap()[t * 128:(t + 1) * 128, :]
                    if t == T // 128 - 1:
                        # last tile: split copies across both engines and DMA
                        # per 512-col chunk so the drain tail is short
                        nc.scalar.copy(out=ot[:, 0:512], in_=o01[:, 0:512])
                        nc.sync.dma_start(out=orow[:, 0:512], in_=ot[:, 0:512])
                        nc.vector.tensor_copy(out=ot[:, 512:1024], in_=o01[:, 512:1024])
                        nc.sync.dma_start(out=orow[:, 512:1024], in_=ot[:, 512:1024])
                        nc.scalar.copy(out=ot[:, 1024:1536], in_=o2)
                        nc.sync.dma_start(out=orow[:, 1024:1536], in_=ot[:, 1024:1536])
                        nc.vector.tensor_copy(out=ot[:, 1536:2048], in_=o3)
                        nc.sync.dma_start(out=orow[:, 1536:2048], in_=ot[:, 1536:2048])
                    else:
                        if t % 2 == 0:
                            nc.scalar.copy(out=ot[:, 0:1024], in_=o01)
                            nc.vector.tensor_copy(out=ot[:, 1024:1536], in_=o2)
                            nc.vector.tensor_copy(out=ot[:, 1536:2048], in_=o3)
                        else:
                            nc.vector.tensor_copy(out=ot[:, 0:1024], in_=o01)
                            nc.scalar.copy(out=ot[:, 1024:1536], in_=o2)
                            nc.scalar.copy(out=ot[:, 1536:2048], in_=o3)
                        nc.sync.dma_start(out=orow[:, 0:1024], in_=ot[:, 0:1024])
                        nc.sync.dma_start(out=orow[:, 1024:2048], in_=ot[:, 1024:2048])
    return nc


_NC = None


def _get_nc():
    global _NC
    if _NC is None:
        _NC = build_nc()
    return _NC


def make_in_maps(x, W_attn, W_proj):
    """Host-side sharding/layout prep. Returns list of 8 per-core input dicts."""
    bf = ml_dtypes.bfloat16
    x2 = np.asarray(x, dtype=np.float32)
    xT = np.ascontiguousarray(np.transpose(x2, (0, 2, 1))).astype(bf)  # [B, C, T]

    W = np.asarray(W_attn, dtype=np.float32)
    Wq, Wk, Wv = W[:, :C], W[:, C:2 * C], W[:, 2 * C:]
    Wp = np.asarray(W_proj, dtype=np.float32)

    # single 128x128 staircase mask: tri[s, q] = (q >= s)
    s_rel = np.arange(128)[:, None]
    q_rel = np.arange(128)[None, :]
    tri = (q_rel >= s_rel).astype(np.float32).astype(bf)  # [128, 128]

    in_maps = []
    for core in range(N_CORES):
        b, hg = core // HPC, core % HPC
        fs = slice(hg * HPC * D, (hg + 1) * HPC * D)   # this core's 512 channels
        wqkv = np.ascontiguousarray(
            np.concatenate([Wq[:, fs], Wk[:, fs], Wv[:, fs]], axis=1)
        ).astype(bf)                                    # [C, 1536]
        wp_l = np.ascontiguousarray(Wp[fs, :]).astype(bf)  # [512, C]
        in_maps.append({
            "xT": np.ascontiguousarray(xT[b]),
            "wqkv": wqkv,
            "wp": wp_l,
            "tri": np.ascontiguousarray(tri),
        })
    return in_maps


def combine_outputs(outs):
    """Sum the 4 per-head-group bf16 partials for each batch; stack to [B, T, C]."""
    out = np.empty((B, T, C), dtype=np.float32)
    for b in range(B):
        acc = outs[b * HPC].astype(np.float32)
        for hg in range(1, HPC):
            acc += outs[b * HPC + hg].astype(np.float32)
        out[b] = acc
    return out


def kernel(x, W_attn, W_proj, mask=None):
    in_maps = make_in_maps(x, W_attn, W_proj)
    nc = _get_nc()
    res = run_bass_kernel_spmd(nc, in_maps, core_ids=list(range(N_CORES)))
    outs = [r["out"] for r in res.results]
    return combine_outputs(outs)


if __name__ == "__main__":
    rng = np.random.default_rng(0)
    x = rng.standard_normal((B, T, C), dtype=np.float32)
    W_attn = rng.standard_normal((C, 3 * C), dtype=np.float32) * 0.02
    W_proj = rng.standard_normal((C, C), dtype=np.float32) * 0.02
    out = kernel(x, W_attn, W_proj)
    print("out", out.shape, out.dtype, np.abs(out).max())
